# revision 1
# baseline (speedup 1.0000x reference)
"""Additive-attention (title/description) kernel, data-parallel over batch
across 8 NeuronCores.

Contract: kernel(**inputs) takes the FULL unsharded inputs
  hiddens_des   (16, 256, 512) f32
  hiddens_title (16, 256, 512) f32
  Wv            (49, 512)      f32
  Wg            (49, 512)      f32
  Wh            (1, 49)        f32
and returns the FULL output (c_hat (16,256,512), alpha (16,256,256)).

Sharding: batch B=16 split 2-per-core across the 8 cores; the tiny
(49xH) weights are replicated. Compute per core:
  cv = des @ Wv^T ; cg = title @ Wg^T
  z[t,k] = sum_a Wh[a] * tanh(cv[k,a] + cg[t,a])
  alpha = softmax_k(z) ; c_hat = alpha @ des
"""

import numpy as np

B, T, K, H, A = 16, 256, 256, 512, 49
NCORE = 8


def _build_pmapped():
    import jax
    import jax.numpy as jnp

    def _attn(des, title, Wv, Wg, Wh):
        # des (b,K,H), title (b,T,H) with b = B // NCORE
        cv = jnp.einsum("bkh,ah->bka", des, Wv)          # (b,K,A)
        cg = jnp.einsum("bth,ah->bta", title, Wg)        # (b,T,A)
        content = cv[:, None, :, :] + cg[:, :, None, :]  # (b,T,K,A)
        z = jnp.einsum("btka,a->btk", jnp.tanh(content), Wh[0])
        alpha = jax.nn.softmax(z, axis=-1)               # softmax over K
        c_hat = jnp.einsum("btk,bkh->bth", alpha, des)   # (b,T,H)
        return c_hat, alpha

    devs = jax.devices()[:NCORE]
    return jax.pmap(_attn, in_axes=(0, 0, None, None, None), devices=devs)


_PMAPPED = None


def kernel(hiddens_des, hiddens_title, Wv, Wg, Wh):
    global _PMAPPED
    des = np.asarray(hiddens_des, dtype=np.float32)
    title = np.asarray(hiddens_title, dtype=np.float32)
    Wv = np.asarray(Wv, dtype=np.float32)
    Wg = np.asarray(Wg, dtype=np.float32)
    Wh = np.asarray(Wh, dtype=np.float32)

    b = B // NCORE  # per-core batch
    des_sh = des.reshape(NCORE, b, K, H)
    title_sh = title.reshape(NCORE, b, T, H)

    try:
        if _PMAPPED is None:
            _PMAPPED = _build_pmapped()
        c_hat_sh, alpha_sh = _PMAPPED(des_sh, title_sh, Wv, Wg, Wh)
        c_hat = np.asarray(c_hat_sh).reshape(B, T, H)
        alpha = np.asarray(alpha_sh).reshape(B, T, K)
    except Exception:
        # Fallback: plain numpy (correctness safety net).
        cv = np.einsum("bkh,ah->bka", des, Wv)
        cg = np.einsum("bth,ah->bta", title, Wg)
        content = cv[:, None, :, :] + cg[:, :, None, :]
        z = np.einsum("btka,a->btk", np.tanh(content), Wh[0])
        zmax = z.max(axis=-1, keepdims=True)
        e = np.exp(z - zmax)
        alpha = e / e.sum(axis=-1, keepdims=True)
        c_hat = np.einsum("btk,bkh->bth", alpha, des)

    return (c_hat.astype(np.float32), alpha.astype(np.float32))
